# revision 16
# baseline (speedup 1.0000x reference)
"""FFM layer (embedding lookup + field factorization) on 8 trn2 NeuronCores.

The reference's inner reduction  latent_sum[b,f,k] = sum_j v[idx[b,f],j,k]
sums over all 26 fields j regardless of the indices, so
  vred[i,k] = sum_j v[i,j,k]        (8 floats)
  c[i]      = w[i] + w0/26 - 0.5*|vred[i]|^2
are pure functions of the parameters, folded into the table host-side.
Device computes  out[b] = sum_f c[idx] + 0.5*|sum_f vred[idx]|^2.

SWDGE dma_gather is descriptor-generation-bound (~7ns/desc on the Q7s =>
~93us/core for the 13312 rows a core needs), so the lookup instead uses the
GPSIMD ap_gather custom instruction on an SBUF-resident table:

 - field-sharded sections: field f's 20000-row sub-table lives on one
   (core, 16-partition group) section, stored "vertically": payload value k
   of row r sits at partition lane k//2, bf16 pair element k%2, free column
   r (d=2).  payload = [vred(8) | c | 0].  26 sections over 8 cores x 4
   groups (2 cores hold 4 fields, 6 hold 3 + a zero dummy).
 - one ap_gather per core: each 16-partition group gathers, for its field,
   all 4096 batch rows in batch order (num_idxs=4096, quota exactly 1 -- no
   dynamic routing anywhere).  8 Q7 cores work their groups in parallel,
   no DMA descriptors at all.
 - cross-group (= cross-field-within-core) reduction via PE matmul with a
   0/1 lane-selector: psum[j2, 2b+m] = sum_groups G[16g+j2, 2b+m] -- the
   only engine that contracts the partition axis.
 - cross-core combine: 8-core AllToAll of per-core partials (each core
   owns 512 batch rows), DVE reduce, then the quadratic tail:
   square, PE ones-reduce over 4 lanes, even/odd add, + c lane.
"""

import sys

import numpy as np

FIELD = 26
K = 8
VOCAB = 20000
TOTAL = FIELD * VOCAB
B = 4096
NCORES = 8
BC = B // NCORES          # 512 batch rows owned per core (output shard)
NG = 4                    # table sections (16-partition groups) per core
NLANE = 5                 # payload lanes used per group (5 bf16 pairs = 10)
NSLOT = B // 16           # 256 idx slots per idx partition
X = 2 * B                 # 8192 interleaved columns (2b+m)

# fields per core: cores 0,1 get 4; cores 2-7 get 3 (+ zero dummy section)
FMAP = [[0, 1, 2, 3], [4, 5, 6, 7]] + [
    [8 + 3 * i, 9 + 3 * i, 10 + 3 * i] for i in range(6)
]

_TRN_REPO = "/opt/trn_rl_repo"

_cache = {}


def _build_nc():
    if _TRN_REPO not in sys.path:
        sys.path.insert(0, _TRN_REPO)
    from concourse import bacc, bass, mybir, tile

    f32 = mybir.dt.float32
    bf16 = mybir.dt.bfloat16
    i16 = mybir.dt.int16
    Alu = mybir.AluOpType
    Ax = mybir.AxisListType

    nc = bacc.Bacc("TRN2", target_bir_lowering=False, debug=False,
                   num_devices=NCORES)

    # tabin[g, j2, r, m] = payload value 2*j2+m of row r of field FMAP[c][g]
    tab_d = nc.dram_tensor("tabin", [NG, NLANE, VOCAB, 2], bf16,
                           kind="ExternalInput")
    # idx16[16g+j, s] = inputs[s*16+j, FMAP[c][g]]
    idx_d = nc.dram_tensor("idx16", [64, NSLOT], i16, kind="ExternalInput")
    # wsel[p, j2] = 1 iff p % 16 == j2  (lane selector for the PE reduce)
    wsel_d = nc.dram_tensor("wsel", [64, NLANE], bf16, kind="ExternalInput")
    # half4 = 0.5*ones over the 4 squared-pair lanes; esel5 selects lane 4
    # (the c lane) -- two accumulating matmuls fold the whole tail reduce
    # into PSUM without any partition-base-4 engine access (mod-32 rule).
    half_d = nc.dram_tensor("half4", [4, 1], bf16, kind="ExternalInput")
    esel_d = nc.dram_tensor("esel5", [NLANE, 1], bf16, kind="ExternalInput")
    out_d = nc.dram_tensor("out", [1, BC], f32, kind="ExternalOutput")

    a2a_in = nc.dram_tensor("a2a_in", [NCORES, NLANE, 2 * BC], f32)
    a2a_out = nc.dram_tensor("a2a_out", [NCORES, NLANE, 2 * BC], f32)

    with tile.TileContext(nc) as tc:
        with tc.tile_pool(name="pool", bufs=1) as pool, \
             tc.tile_pool(name="ps", bufs=1,
                          space=bass.MemorySpace.PSUM) as pspool:
            idx_sb = pool.tile([64, NSLOT], i16, tag="idx")
            nc.sync.dma_start(out=idx_sb[:], in_=idx_d[:, :])
            wsel_sb = pool.tile([64, NLANE], bf16, tag="wsel")
            nc.sync.dma_start(out=wsel_sb[:], in_=wsel_d[:, :])
            half_sb = pool.tile([4, 1], bf16, tag="half")
            nc.sync.dma_start(out=half_sb[:], in_=half_d[:, :])
            esel_sb = pool.tile([NLANE, 1], bf16, tag="esel")
            nc.sync.dma_start(out=esel_sb[:], in_=esel_d[:, :])

            # vertical table: partition 16g+j2 holds bf16 pair (2j2, 2j2+1)
            # of section g's rows; lanes 5..15 stay garbage (never selected)
            tab_sb = pool.tile([64, VOCAB, 2], bf16, tag="tab")
            # lanes NLANE..15 are never DMA'd; zero the tile first so PE's
            # 0-weight contraction can't hit NaN garbage (0*NaN=NaN)
            nc.vector.memset(tab_sb[:], 0)
            eng = [nc.sync, nc.scalar, nc.gpsimd, nc.sync]
            for g in range(NG):
                eng[g].dma_start(
                    out=tab_sb[16 * g:16 * g + NLANE, :, :],
                    in_=tab_d[g, :, :, :],
                )

            # G[16g+j2, b, m] = tab[16g+j2, idx[b, f_g], m]
            gth = pool.tile([64, B, 2], bf16, tag="g")
            nc.gpsimd.ap_gather(
                out_ap=gth[:],
                in_ap=tab_sb[:],
                idxs_ap=idx_sb[:],
                channels=64,
                num_elems=VOCAB,
                d=2,
                num_idxs=B,
            )
            gflat = gth[:].rearrange("p b m -> p (b m)")  # [64, X]

            # cross-section reduce: part[j2, x] = sum_g G[16g+j2, x]
            part = pool.tile([NLANE, X], f32, tag="part")
            for r in range(2):
                acc = pspool.tile([NLANE, X // 2], f32, tag="acc")
                for k in range(8):
                    o = 512 * k
                    nc.tensor.matmul(
                        acc[:, o:o + 512],
                        wsel_sb[:],
                        gflat[:, X // 2 * r + o:X // 2 * r + o + 512],
                        start=True,
                        stop=True,
                    )
                nc.vector.tensor_copy(
                    part[:, X // 2 * r:X // 2 * (r + 1)], acc[:]
                )

            # exchange: chunk c2 = partials for batch rows [512*c2, 512*c2+512)
            nc.scalar.dma_start(
                out=a2a_in[:, :, :].rearrange("c j x -> j c x"),
                in_=part[:].rearrange("p (c x) -> p c x", c=NCORES),
            )
            nc.gpsimd.collective_compute(
                "AllToAll",
                Alu.bypass,
                replica_groups=[list(range(NCORES))],
                ins=[a2a_in[:, :, :]],
                outs=[a2a_out[:, :, :]],
            )
            q = pool.tile([NLANE, NCORES, 2 * BC], f32, tag="q")
            nc.sync.dma_start(
                out=q[:], in_=a2a_out[:, :, :].rearrange("c j x -> j c x")
            )
            red = pool.tile([NLANE, 2 * BC], f32, tag="red")
            nc.vector.tensor_reduce(
                out=red[:], in_=q[:].rearrange("p c x -> p x c"),
                axis=Ax.X, op=Alu.add,
            )

            # tail: out[b] = red[4, 2b] + 0.5 * sum_{j2<4,m} red[j2, 2b+m]^2
            # psum[0, x] = 0.5*sum_{j2<4} red[j2,x]^2 + red[4,x] via two
            # accumulating matmuls; then even+odd pair-collapse is the answer
            # (red[4, odd] == 0 by table padding).
            sqb = pool.tile([4, 2 * BC], bf16, tag="sqb")
            nc.vector.tensor_tensor(
                out=sqb[:], in0=red[0:4], in1=red[0:4], op=Alu.mult
            )
            redb = pool.tile([NLANE, 2 * BC], bf16, tag="redb")
            nc.vector.tensor_copy(redb[:], red[:])
            ps2full = pspool.tile([NLANE, X // 2], f32, tag="acc")
            ps2 = ps2full[0:1, 0:2 * BC]
            for k in range(2):
                nc.tensor.matmul(
                    ps2[:, 512 * k:512 * k + 512],
                    half_sb[:],
                    sqb[:, 512 * k:512 * k + 512],
                    start=True,
                    stop=False,
                )
                nc.tensor.matmul(
                    ps2[:, 512 * k:512 * k + 512],
                    esel_sb[:],
                    redb[:, 512 * k:512 * k + 512],
                    start=False,
                    stop=True,
                )
            s2pair = pool.tile([1, 2 * BC], f32, tag="s2p")
            nc.vector.tensor_copy(s2pair[:], ps2[:])

            s2even = s2pair[:].rearrange("p (b two) -> p b two", two=2)
            res = pool.tile([1, BC], f32, tag="res")
            nc.vector.tensor_tensor(
                out=res[:], in0=s2even[:, :, 0], in1=s2even[:, :, 1],
                op=Alu.add,
            )
            nc.sync.dma_start(out=out_d[:, :], in_=res[:])
    nc.compile()
    return nc


def get_nc():
    if "nc" not in _cache:
        _cache["nc"] = _build_nc()
    return _cache["nc"]


def make_in_maps(inputs, offsets, w0, w, v):
    del offsets  # folded into the per-field sections
    import ml_dtypes

    bf16 = ml_dtypes.bfloat16
    inp = np.asarray(inputs).astype(np.int16)        # [B, FIELD], < 20000
    vred = np.asarray(v, np.float32).reshape(TOTAL, FIELD, K).sum(axis=1)
    cval = (np.asarray(w, np.float32).reshape(TOTAL)
            + np.float32(np.asarray(w0, np.float32).reshape(()) / FIELD)
            - 0.5 * (vred * vred).sum(axis=1))

    wsel = np.zeros((64, NLANE), dtype=bf16)
    for p in range(64):
        if p % 16 < NLANE:
            wsel[p, p % 16] = 1
    half4 = np.full((4, 1), 0.5, dtype=bf16)
    esel5 = np.zeros((NLANE, 1), dtype=bf16)
    esel5[4, 0] = 1

    maps = []
    for c in range(NCORES):
        fields = FMAP[c]
        tabin = np.zeros((NG, NLANE, VOCAB, 2), dtype=bf16)
        idx16 = np.zeros((64, NSLOT), dtype=np.int16)
        for g, f in enumerate(fields):
            pay = np.zeros((VOCAB, 2 * NLANE), dtype=np.float32)
            pay[:, :K] = vred[f * VOCAB:(f + 1) * VOCAB]
            pay[:, K] = cval[f * VOCAB:(f + 1) * VOCAB]
            tabin[g] = pay.reshape(VOCAB, NLANE, 2).transpose(1, 0, 2)
            # idx16[16g+j, s] = inputs[s*16+j, f]
            idx16[16 * g:16 * g + 16] = inp[:, f].reshape(NSLOT, 16).T
        maps.append({"tabin": tabin, "idx16": idx16, "wsel": wsel,
                     "half4": half4, "esel5": esel5})
    return maps


def assemble_out(res):
    return np.concatenate(
        [np.asarray(res.results[i]["out"]).reshape(BC)
         for i in range(NCORES)]
    ).reshape(B, 1).astype(np.float32)


def kernel(inputs, offsets, w0, w, v):
    if _TRN_REPO not in sys.path:
        sys.path.insert(0, _TRN_REPO)
    from concourse.bass_utils import run_bass_kernel_spmd

    nc = get_nc()
    in_maps = make_in_maps(inputs, offsets, w0, w, v)
    res = run_bass_kernel_spmd(nc, in_maps, list(range(NCORES)))
    return assemble_out(res)


# revision 17
# speedup vs baseline: 1.3630x; 1.3630x over previous
"""FFM layer (embedding lookup + field factorization) on 8 trn2 NeuronCores.

The reference's inner reduction  latent_sum[b,f,k] = sum_j v[idx[b,f],j,k]
sums over all 26 fields j regardless of the indices, so
  vred[i,k] = sum_j v[i,j,k]        (8 floats)
  c[i]      = w[i] + w0/26 - 0.5*|vred[i]|^2
are pure functions of the parameters, folded into the table host-side.
Device computes  out[b] = sum_f c[idx] + 0.5*|sum_f vred[idx]|^2.

SWDGE dma_gather is descriptor-generation-bound (~7ns/desc on the Q7s =>
~93us/core for the 13312 rows a core needs), so the lookup instead uses the
GPSIMD ap_gather custom instruction on an SBUF-resident table (no DMA
descriptors at all):

 - sections = (field, batch-half): each section owns field f's 20000-row
   sub-table, stored "vertically": payload value k of row r sits at
   partition lane k//2, bf16 pair element k%2, free column r (d=2).
   payload = [vred(8) | c | 0].  8 sections per core (one per 16-partition
   group, so all 8 Q7 cores work in parallel), 2048 batch rows each.
   2 cores hold 4 fields, 6 hold 3 (+2 zero dummy sections).
 - one ap_gather per core: group 2*fl+h gathers, for its field, batch rows
   [2048h, 2048h+2048) in batch order (quota exactly 1 -- no routing).
 - cross-field reduce via PE matmul (the only partition-axis contractor)
   with a 0/1 (half, lane) selector -> psum partials [10, 4096].
 - cross-core combine: 8-core AllToAll of bf16 partials (each core owns
   512 batch rows), PE source-reduce over 40 stacked partitions, then the
   quadratic tail folded into two accumulating matmuls.
"""

import sys

import numpy as np

FIELD = 26
K = 8
VOCAB = 20000
TOTAL = FIELD * VOCAB
B = 4096
NCORES = 8
BC = B // NCORES          # 512 batch rows owned per core (output shard)
BH = B // 2               # 2048 rows per batch half / per section
SEC = 8                   # sections (16-partition groups) per core
NLANE = 5                 # payload lanes used per group (5 bf16 pairs)
NSLOT = BH // 16          # 128 idx slots per idx partition

# fields per core: cores 0,1 get 4; cores 2-7 get 3 (+ zero dummy)
FMAP = [[0, 1, 2, 3], [4, 5, 6, 7]] + [
    [8 + 3 * i, 9 + 3 * i, 10 + 3 * i] for i in range(6)
]

_TRN_REPO = "/opt/trn_rl_repo"

_cache = {}


def _build_nc():
    if _TRN_REPO not in sys.path:
        sys.path.insert(0, _TRN_REPO)
    from concourse import bacc, bass, mybir, tile

    f32 = mybir.dt.float32
    bf16 = mybir.dt.bfloat16
    i16 = mybir.dt.int16
    Alu = mybir.AluOpType

    nc = bacc.Bacc("TRN2", target_bir_lowering=False, debug=False,
                   num_devices=NCORES)

    # tabin[g, j2, r, m] = payload value 2*j2+m of row r of section g's field
    tab_d = nc.dram_tensor("tabin", [SEC, NLANE, VOCAB, 2], bf16,
                           kind="ExternalInput")
    # idx16[16g+j, s] = inputs[2048*(g%2) + s*16+j, field(g)]
    idx_d = nc.dram_tensor("idx16", [128, NSLOT], i16, kind="ExternalInput")
    # wsel[p, 5h+j2] = 1 iff (p%32)//16 == h and p%16 == j2
    wsel_d = nc.dram_tensor("wsel", [128, 2 * NLANE], bf16,
                            kind="ExternalInput")
    # sumsel[5s+j, j2] = 1 iff j == j2 (8-source post-A2A reduce)
    ssel_d = nc.dram_tensor("sumsel", [40, NLANE], bf16,
                            kind="ExternalInput")
    half_d = nc.dram_tensor("half4", [4, 1], bf16, kind="ExternalInput")
    esel_d = nc.dram_tensor("esel5", [NLANE, 1], bf16, kind="ExternalInput")
    zl_d = nc.dram_tensor("zlane", [11, BH, 2], bf16, kind="ExternalInput")
    out_d = nc.dram_tensor("out", [1, BC], f32, kind="ExternalOutput")

    a2a_in = nc.dram_tensor("a2a_in", [NCORES, NLANE, 2 * BC], bf16)
    a2a_out = nc.dram_tensor("a2a_out", [NCORES, NLANE, 2 * BC], bf16)

    with tile.TileContext(nc) as tc:
        with tc.tile_pool(name="pool", bufs=1) as pool, \
             tc.tile_pool(name="ps", bufs=1,
                          space=bass.MemorySpace.PSUM) as pspool:
            idx_sb = pool.tile([128, NSLOT], i16, tag="idx")
            nc.sync.dma_start(out=idx_sb[:], in_=idx_d[:, :])
            wsel_sb = pool.tile([128, 2 * NLANE], bf16, tag="wsel")
            nc.sync.dma_start(out=wsel_sb[:], in_=wsel_d[:, :])
            ssel_sb = pool.tile([40, NLANE], bf16, tag="ssel")
            nc.sync.dma_start(out=ssel_sb[:], in_=ssel_d[:, :])
            half_sb = pool.tile([4, 1], bf16, tag="half")
            nc.sync.dma_start(out=half_sb[:], in_=half_d[:, :])
            esel_sb = pool.tile([NLANE, 1], bf16, tag="esel")
            nc.sync.dma_start(out=esel_sb[:], in_=esel_d[:, :])
            zl_sb = pool.tile([11, BH, 2], bf16, tag="zl")
            nc.scalar.dma_start(out=zl_sb[:], in_=zl_d[:, :, :])

            # vertical table: partition 16g+j2 holds bf16 pair (2j2, 2j2+1)
            # of section g's rows; lanes 5..15 unloaded (zeroed in G below)
            tab_sb = pool.tile([128, VOCAB, 2], bf16, tag="tab")
            eng = [nc.sync, nc.scalar, nc.gpsimd, nc.sync,
                   nc.scalar, nc.gpsimd, nc.sync, nc.scalar]
            for g in range(SEC):
                eng[g].dma_start(
                    out=tab_sb[16 * g:16 * g + NLANE, :, :],
                    in_=tab_d[g, :, :, :],
                )

            # G[16g+j2, b', m] = tab[16g+j2, idx[b', f_g], m]
            gth = pool.tile([128, BH, 2], bf16, tag="g")
            nc.gpsimd.ap_gather(
                out_ap=gth[:],
                in_ap=tab_sb[:],
                idxs_ap=idx_sb[:],
                channels=128,
                num_elems=VOCAB,
                d=2,
                num_idxs=BH,
            )
            # zero G's garbage lanes so PE's 0-weight contraction can't
            # hit NaN garbage (0*NaN=NaN); DMA is exempt from the mod-32
            # partition-base rule
            for g in range(SEC):
                eng[g].dma_start(
                    out=gth[16 * g + NLANE:16 * (g + 1), :, :],
                    in_=zl_sb[:],
                )
            gflat = gth[:].rearrange("p b m -> p (b m)")  # [128, 4096]

            # cross-field reduce: acc[5h+j2, x] = sum_fl G[32fl+16h+j2, x]
            acc = pspool.tile([2 * NLANE, 2 * BH], f32, tag="acc")
            for k in range(8):
                o = 512 * k
                nc.tensor.matmul(
                    acc[:, o:o + 512],
                    wsel_sb[:],
                    gflat[:, o:o + 512],
                    start=True,
                    stop=True,
                )
            pb = pool.tile([2 * NLANE, 2 * BH], bf16, tag="pb")
            nc.vector.tensor_copy(pb[:], acc[:])

            # exchange: chunk c2=(4h+q) = partials for rows 2048h+512q..+512
            for h in range(2):
                nc.scalar.dma_start(
                    out=a2a_in[4 * h:4 * h + 4, :, :]
                    .rearrange("c j x -> j c x"),
                    in_=pb[NLANE * h:NLANE * (h + 1)]
                    .rearrange("p (c x) -> p c x", c=4),
                )
            nc.gpsimd.collective_compute(
                "AllToAll",
                Alu.bypass,
                replica_groups=[list(range(NCORES))],
                ins=[a2a_in[:, :, :]],
                outs=[a2a_out[:, :, :]],
            )
            q = pool.tile([40, 2 * BC], bf16, tag="q")
            nc.sync.dma_start(
                out=q[:], in_=a2a_out[:, :, :].rearrange("c j x -> (c j) x")
            )
            # 8-source reduce on PE: red[j2, x] = sum_s q[5s+j2, x]
            acc2 = pspool.tile([2 * NLANE, 2 * BH], f32, tag="acc")
            accr = acc2[0:NLANE, 0:2 * BC]
            for k in range(2):
                nc.tensor.matmul(
                    accr[:, 512 * k:512 * k + 512],
                    ssel_sb[:],
                    q[:, 512 * k:512 * k + 512],
                    start=True,
                    stop=True,
                )
            red = pool.tile([NLANE, 2 * BC], f32, tag="red")
            nc.vector.tensor_copy(red[:], accr[:])

            # tail: psum[0, x] = 0.5*sum_{j2<4} red[j2,x]^2 + red[4,x]
            # (red[4, odd] == 0 by table padding), then even+odd collapse
            sqb = pool.tile([4, 2 * BC], bf16, tag="sqb")
            nc.vector.tensor_tensor(
                out=sqb[:], in0=red[0:4], in1=red[0:4], op=Alu.mult
            )
            redb = pool.tile([NLANE, 2 * BC], bf16, tag="redb")
            nc.vector.tensor_copy(redb[:], red[:])
            acc3 = pspool.tile([2 * NLANE, 2 * BH], f32, tag="acc")
            ps2 = acc3[0:1, 0:2 * BC]
            for k in range(2):
                nc.tensor.matmul(
                    ps2[:, 512 * k:512 * k + 512],
                    half_sb[:],
                    sqb[:, 512 * k:512 * k + 512],
                    start=True,
                    stop=False,
                )
                nc.tensor.matmul(
                    ps2[:, 512 * k:512 * k + 512],
                    esel_sb[:],
                    redb[:, 512 * k:512 * k + 512],
                    start=False,
                    stop=True,
                )
            s2pair = pool.tile([1, 2 * BC], f32, tag="s2p")
            nc.vector.tensor_copy(s2pair[:], ps2[:])

            s2even = s2pair[:].rearrange("p (b two) -> p b two", two=2)
            res = pool.tile([1, BC], f32, tag="res")
            nc.vector.tensor_tensor(
                out=res[:], in0=s2even[:, :, 0], in1=s2even[:, :, 1],
                op=Alu.add,
            )
            nc.sync.dma_start(out=out_d[:, :], in_=res[:])
    nc.compile()
    return nc


def get_nc():
    if "nc" not in _cache:
        _cache["nc"] = _build_nc()
    return _cache["nc"]


def make_in_maps(inputs, offsets, w0, w, v):
    del offsets  # folded into the per-field sections
    import ml_dtypes

    bf16 = ml_dtypes.bfloat16
    inp = np.asarray(inputs).astype(np.int16)        # [B, FIELD], < 20000
    vred = np.asarray(v, np.float32).reshape(TOTAL, FIELD, K).sum(axis=1)
    cval = (np.asarray(w, np.float32).reshape(TOTAL)
            + np.float32(np.asarray(w0, np.float32).reshape(()) / FIELD)
            - 0.5 * (vred * vred).sum(axis=1))

    wsel = np.zeros((128, 2 * NLANE), dtype=bf16)
    for p in range(128):
        h, j = (p % 32) // 16, p % 16
        if j < NLANE:
            wsel[p, NLANE * h + j] = 1
    ssel = np.zeros((40, NLANE), dtype=bf16)
    for p in range(40):
        ssel[p, p % NLANE] = 1
    half4 = np.full((4, 1), 0.5, dtype=bf16)
    esel5 = np.zeros((NLANE, 1), dtype=bf16)
    esel5[4, 0] = 1
    zlane = np.zeros((11, BH, 2), dtype=bf16)

    maps = []
    for c in range(NCORES):
        fields = FMAP[c]
        tabin = np.zeros((SEC, NLANE, VOCAB, 2), dtype=bf16)
        idx16 = np.zeros((128, NSLOT), dtype=np.int16)
        for fl, f in enumerate(fields):
            pay = np.zeros((VOCAB, 2 * NLANE), dtype=np.float32)
            pay[:, :K] = vred[f * VOCAB:(f + 1) * VOCAB]
            pay[:, K] = cval[f * VOCAB:(f + 1) * VOCAB]
            vert = pay.reshape(VOCAB, NLANE, 2).transpose(1, 0, 2)
            for h in range(2):
                g = 2 * fl + h
                tabin[g] = vert
                # idx16[16g+j, s] = inputs[2048h + s*16+j, f]
                idx16[16 * g:16 * g + 16] = (
                    inp[BH * h:BH * (h + 1), f].reshape(NSLOT, 16).T
                )
        maps.append({"tabin": tabin, "idx16": idx16, "wsel": wsel,
                     "sumsel": ssel, "half4": half4, "esel5": esel5,
                     "zlane": zlane})
    return maps


def assemble_out(res):
    return np.concatenate(
        [np.asarray(res.results[i]["out"]).reshape(BC)
         for i in range(NCORES)]
    ).reshape(B, 1).astype(np.float32)


def kernel(inputs, offsets, w0, w, v):
    if _TRN_REPO not in sys.path:
        sys.path.insert(0, _TRN_REPO)
    from concourse.bass_utils import run_bass_kernel_spmd

    nc = get_nc()
    in_maps = make_in_maps(inputs, offsets, w0, w, v)
    res = run_bass_kernel_spmd(nc, in_maps, list(range(NCORES)))
    return assemble_out(res)


# revision 20
# speedup vs baseline: 1.8426x; 1.3518x over previous
"""FFM layer (embedding lookup + field-factorization) on 8 trn2 NeuronCores.

Strategy: data-parallel over batch (4096 rows -> 512/core), embedding tables
replicated to every core.  The reference's inner reduction
  latent_sum[b,f,k] = sum_j v[idx[b,f], j, k]
sums over ALL 26 fields j regardless of the batch indices, so
  vred[i,k] = sum_j v[i,j,k]
is a pure function of the parameters and is folded into the table host-side
(same spirit as packing w into the augmented table).  Likewise the
second-order self term and first-order weight fold into one row scalar
  c[i] = w[i] + w0/26 - 0.5*|vred[i]|^2,
leaving the device with
  out[b] = sum_f c[idx[b,f]] + 0.5 * |sum_f vred[idx[b,f]]|^2.

Each table row is [vred (8 f32) | c | pad] = 64 f32 = 256 B, the SWDGE
minimum elem size -- 4x fewer gathered bytes than the 1 KiB rows of the
naive packing, and no on-device j-reduction at all.  Lookups use the SWDGE
dma_gather custom instruction, one per field (field-local int16 indices,
512 per gather).  Index ordinal i = batch row lands at dest
[i % 128, i // 128, :], exactly the (partition, batch-tile) layout the
VectorE tail wants.
"""

import sys

import numpy as np

FIELD = 26
K = 8
RPAD = 64                # padded row length in f32 (256 B, SWDGE minimum)
VOCAB = 20000
TOTAL = FIELD * VOCAB    # 520000
B = 4096
NCORES = 8
BC = B // NCORES         # 512 batch rows per core
P = 128
NTILES = BC // P         # 4
NSLOT = BC // 16         # 32 int16 index slots per idx partition

_TRN_REPO = "/opt/trn_rl_repo"

_cache = {}


def _build_nc(n_iters=1):
    if _TRN_REPO not in sys.path:
        sys.path.insert(0, _TRN_REPO)
    from concourse import bacc, mybir, tile

    f32 = mybir.dt.float32
    i16 = mybir.dt.int16
    Alu = mybir.AluOpType
    Ax = mybir.AxisListType

    nc = bacc.Bacc("TRN2", target_bir_lowering=False, debug=False)
    # idx16[p, f, s] = int16 field-local index of batch row s*16+(p%16),
    # field f -- 16-partition wrap replicated to 128 host-side
    idx_d = nc.dram_tensor("idx16", [P, FIELD, NSLOT], i16,
                           kind="ExternalInput")
    tab_d = nc.dram_tensor("tab", [TOTAL, RPAD], f32, kind="ExternalInput")
    out_d = nc.dram_tensor("out", [BC, 1], f32, kind="ExternalOutput")

    with tile.TileContext(nc) as tc:
        with tc.tile_pool(name="pool", bufs=1) as pool:
            for _ in range(n_iters):
                idx_sb = pool.tile([P, FIELD, NSLOT], i16, tag="idx")
                # field 0's indices land first so its gather can issue
                # while the rest of the idx tile streams in
                nc.sync.dma_start(out=idx_sb[:, 0:1, :], in_=idx_d[:, 0:1, :])
                nc.scalar.dma_start(out=idx_sb[:, 1:, :], in_=idx_d[:, 1:, :])
                # tiny dummy gather absorbs the one-time SWDGE init
                # concurrently with the idx upload
                dum_i = pool.tile([P, 1], i16, tag="dmi")
                nc.vector.memset(dum_i[:], 0)
                dum_o = pool.tile([P, 1, RPAD], f32, tag="dmo")
                nc.gpsimd.dma_gather(
                    out_ap=dum_o[:],
                    in_ap=tab_d[0:VOCAB, :],
                    idxs_ap=dum_i[:],
                    num_idxs=16,
                    num_idxs_reg=16,
                    elem_size=RPAD,
                )

                # vg[p, f, t, :] = tab[f*VOCAB + idx[t*128+p, f], :]
                vg = pool.tile([P, FIELD, NTILES, RPAD], f32, tag="vg")
                for f in range(FIELD):
                    nc.gpsimd.dma_gather(
                        out_ap=vg[:, f],
                        in_ap=tab_d[f * VOCAB:(f + 1) * VOCAB, :],
                        idxs_ap=idx_sb[:, f, :],
                        num_idxs=BC,
                        num_idxs_reg=BC,
                        elem_size=RPAD,
                    )

                # s[p, t, k] = sum_f vred[idx, k]
                s_all = pool.tile([P, NTILES, K], f32, tag="s")
                nc.vector.tensor_reduce(
                    out=s_all[:],
                    in_=vg[:, :, :, 0:K].rearrange("p f t k -> p t k f"),
                    axis=Ax.X,
                    op=Alu.add,
                )
                # csum[p, t] = sum_f c[idx]
                csum = pool.tile([P, NTILES], f32, tag="c")
                nc.vector.tensor_reduce(
                    out=csum[:],
                    in_=vg[:, :, :, K].rearrange("p f t -> p t f"),
                    axis=Ax.X,
                    op=Alu.add,
                )
                ssq = pool.tile([P, NTILES, K], f32, tag="ssq")
                nc.vector.tensor_tensor(
                    out=ssq[:], in0=s_all[:], in1=s_all[:], op=Alu.mult
                )
                s2 = pool.tile([P, NTILES], f32, tag="s2")
                nc.vector.tensor_reduce(
                    out=s2[:], in_=ssq[:], axis=Ax.X, op=Alu.add
                )
                s2h = pool.tile([P, NTILES], f32, tag="s2h")
                nc.vector.tensor_scalar_mul(s2h[:], s2[:], 0.5)
                out_all = pool.tile([P, NTILES], f32, tag="oa")
                nc.vector.tensor_tensor(
                    out=out_all[:], in0=s2h[:], in1=csum[:], op=Alu.add
                )
                # single store: out[t*128+p] = out_all[p, t]
                nc.sync.dma_start(
                    out=out_d[:, :].rearrange("(t p) one -> p (t one)", p=P),
                    in_=out_all[:],
                )
    nc.compile()
    return nc


def get_nc():
    if "nc" not in _cache:
        _cache["nc"] = _build_nc()
    return _cache["nc"]


def make_in_maps(inputs, offsets, w0, w, v):
    del offsets  # folded into the per-field subtable slicing
    inp = np.asarray(inputs)
    idx16 = np.ascontiguousarray(
        inp.astype(np.int16).reshape(NCORES, BC, FIELD)
    )
    # reduced table row: [vred (8 f32) | c | pad to 64 f32 = 256 B]
    vred = np.asarray(v, dtype=np.float32).reshape(TOTAL, FIELD, K).sum(axis=1)
    c = (np.asarray(w, dtype=np.float32).reshape(TOTAL)
         + np.float32(np.asarray(w0, np.float32).reshape(()) / FIELD)
         - 0.5 * (vred * vred).sum(axis=1))
    tab = np.zeros((TOTAL, RPAD), dtype=np.float32)
    tab[:, :K] = vred
    tab[:, K] = c
    maps = []
    for i in range(NCORES):
        shard = idx16[i]                       # [BC, FIELD]
        wrapped = shard.reshape(NSLOT, 16, FIELD).transpose(1, 2, 0)
        # [16, FIELD, NSLOT] -> replicate to 128 partitions
        rep = np.ascontiguousarray(np.tile(wrapped, (NCORES, 1, 1)))
        maps.append({"idx16": rep, "tab": tab})
    return maps


def assemble_out(res):
    return np.concatenate(
        [np.asarray(res.results[i]["out"]) for i in range(NCORES)], axis=0
    ).astype(np.float32)


def kernel(inputs, offsets, w0, w, v):
    if _TRN_REPO not in sys.path:
        sys.path.insert(0, _TRN_REPO)
    from concourse.bass_utils import run_bass_kernel_spmd

    nc = get_nc()
    in_maps = make_in_maps(inputs, offsets, w0, w, v)
    res = run_bass_kernel_spmd(nc, in_maps, list(range(NCORES)))
    return assemble_out(res)
